# revision 30
# baseline (speedup 1.0000x reference)
"""Trainium2 Bass kernel for single-head causal attention with projections.

Reference computation (B=4, T=4096, D=1024, H=64):
    qh = q @ Wq; kh = k @ Wk; vh = v @ Wv          # [B,T,H]
    S  = qh @ kh.T / sqrt(H)  (causal masked)       # [B,T,T]
    out = softmax(S) @ vh                           # [B,T,H]

Sharding: 8 cores = 4 batches x 2 KV-parity halves. Each core owns its
batch's FULL q rows and the alternating 128-wide KV chunks of one parity,
so causal work balances exactly and no projection work is duplicated
within a core pair (q proj is duplicated instead of k+v, which is
cheaper). Cores return unnormalized partial attention accumulators
PV^T [H+1, T] (ones-column = exp-sum denominators); the host adds the
two parity halves per batch and normalizes - removing all on-device
transposes/reciprocals at the kernel tail and keeping a single big
output DMA.

All matmuls run in bf16 (moving-operand cost 1 cycle/column; fp32r was
1.24x slower on HW and fp8 fails the accuracy budget). Scores compute in
"ST orientation" (kv on partitions, q free) so exp(S^T) feeds the PV
matmul directly. No running max: scores are O(5) for this data regime.
Diagonal-chunk causal masks are two constant [128, 512] patterns
(group-invariant), multiplied in after exp.
"""

import numpy as np

B, T, D, H = 4, 4096, 1024, 64
DC = D // 128        # d chunks
NG = T // 512        # q groups of 512 rows
NO = T // 256        # owned kv chunks per core (16 of 32, alternating)

_CACHE = {}


def _build_program(counts, apply_mask):
    import concourse.bacc as bacc
    import concourse.mybir as mybir
    import concourse.tile as tile
    from concourse.masks import make_identity

    f32 = mybir.dt.float32
    bf16 = mybir.dt.bfloat16

    nc = bacc.Bacc(None, target_bir_lowering=False, debug=False)
    qT = nc.declare_dram_parameter("qT", [128, DC, T], bf16, isOutput=False)
    kT = nc.declare_dram_parameter("kT", [128, DC, NO * 128], bf16,
                                   isOutput=False)
    vT = nc.declare_dram_parameter("vT", [128, DC, NO * 128], bf16,
                                   isOutput=False)
    wq = nc.declare_dram_parameter("wq", [128, DC, H], bf16, isOutput=False)
    wk = nc.declare_dram_parameter("wk", [128, DC, H], bf16, isOutput=False)
    wv = nc.declare_dram_parameter("wv", [128, DC, H], bf16, isOutput=False)
    if apply_mask:
        dmask = nc.declare_dram_parameter("dmask", [128, 2, 512], bf16,
                                          isOutput=False)
    out = nc.declare_dram_parameter("out", [H + 1, T], f32, isOutput=True)

    scale = 1.0 / float(np.sqrt(H))

    with tile.TileContext(nc) as tc:
        with (
            tc.tile_pool(name="singles", bufs=1) as singles,
            tc.tile_pool(name="qstream", bufs=3) as qstream,
            tc.tile_pool(name="kvstream", bufs=2) as kvstream,
            tc.tile_pool(name="work", bufs=4) as work,
            tc.tile_pool(name="proj_ps", bufs=3, space="PSUM") as pps,
            tc.tile_pool(name="st_ps", bufs=2, space="PSUM") as stps,
            tc.tile_pool(name="ptr_ps", bufs=1, space="PSUM") as ptrps,
            tc.tile_pool(name="pvt_ps", bufs=1, space="PSUM") as pvtps,
        ):
            wq_sb = singles.tile([128, DC, H], bf16, tag="wq")
            wk_sb = singles.tile([128, DC, H], bf16, tag="wk")
            wv_sb = singles.tile([128, DC, H], bf16, tag="wv")
            nc.sync.dma_start(out=wq_sb, in_=wq[:, :, :])

            khT = singles.tile([64, NO * 128], bf16, tag="khT")
            vh1 = singles.tile([128, NO, H + 1], bf16, tag="vh1")

            # ---- startup: spread DMA issues over sync/scalar/gpsimd so
            # issue serialization (~650ns each) doesn't gate group 0; 1KB
            # line strips self-limit DMA rate, which keeps the HAM power
            # throttle mostly away (2KB max-rate bursts trip it) ----
            qt0 = qstream.tile([128, DC, 1024], bf16, tag="qt")
            for c in range(0, DC, 2):  # group-0 q, consumption-ordered
                nc.sync.dma_start(out=qt0[:, c:c + 2, 0:512],
                                  in_=qT[:, c:c + 2, 0:512])
            kt0 = kvstream.tile([128, DC, 1024], bf16, tag="kt")
            vt0 = kvstream.tile([128, DC, 1024], bf16, tag="vt")
            for c in range(0, DC, 2):  # groups 0-1 k then v, on scalar queue
                nc.scalar.dma_start(out=kt0[:, c:c + 2, 0:512],
                                    in_=kT[:, c:c + 2, 0:512])
            for c in range(0, DC, 2):
                nc.scalar.dma_start(out=vt0[:, c:c + 2, 0:512],
                                    in_=vT[:, c:c + 2, 0:512])
            # small singles on gpsimd so they don't occupy the sync queue
            nc.gpsimd.dma_start(out=wk_sb, in_=wk[:, :, :])
            nc.gpsimd.dma_start(out=wv_sb, in_=wv[:, :, :])
            if apply_mask:
                dm_sb = singles.tile([128, 2, 512], bf16, tag="dm")
                nc.gpsimd.dma_start(out=dm_sb, in_=dmask[:, :, :])

            # remaining input bulk: all on the sync HW-DGE queue, issued
            # back-to-back in consumption order; issue ops block on ring
            # backpressure so sync becomes a pipelined feeder that stays
            # ahead of the PE
            nc.sync.dma_start(out=qt0[:, :, 512:1024],
                              in_=qT[:, :, 512:1024])
            nc.sync.dma_start(out=kt0[:, :, 512:1024],
                              in_=kT[:, :, 512:1024])
            nc.sync.dma_start(out=vt0[:, :, 512:1024],
                              in_=vT[:, :, 512:1024])
            qts = [qt0]
            kts = {0: kt0}
            vts = {0: vt0}
            for g in (1,):
                qt = qstream.tile([128, DC, 1024], bf16, tag="qt")
                qts.append(qt)
                nc.sync.dma_start(
                    out=qt, in_=qT[:, :, 1024 * g:1024 * g + 1024])
            kt1 = kvstream.tile([128, DC, 1024], bf16, tag="kt")
            vt1 = kvstream.tile([128, DC, 1024], bf16, tag="vt")
            kts[1], vts[1] = kt1, vt1
            nc.sync.dma_start(out=kt1, in_=kT[:, :, 1024:2048])
            nc.sync.dma_start(out=vt1, in_=vT[:, :, 1024:2048])
            for g in (2, 3):
                qt = qstream.tile([128, DC, 1024], bf16, tag="qt")
                qts.append(qt)
                nc.sync.dma_start(
                    out=qt, in_=qT[:, :, 1024 * g:1024 * g + 1024])

            identf = singles.tile([128, 128], f32, tag="identf")
            make_identity(nc, identf)
            identb = singles.tile([128, 128], bf16, tag="identb")
            nc.vector.tensor_copy(identb, identf)
            nc.vector.memset(vh1[:, :, H:H + 1], 1.0)

            for j in range(NG):
                qt = qts[j // 2]
                qcol = (j % 2) * 512

                # ---- q projection for this group ----
                psq = pps.tile([64, 512], f32, tag="ps")
                for c in range(DC):
                    nc.tensor.matmul(psq, wq_sb[:, c, :],
                                     qt[:, c, qcol:qcol + 512],
                                     start=(c == 0), stop=(c == DC - 1))
                qh = work.tile([64, 512], bf16, tag="qh")
                nc.vector.tensor_copy(qh, psq)

                # ---- k/v projections, two groups at a time (512 cols) ----
                if j % 2 == 0:
                    kt, vt = kts[j // 4], vts[j // 4]
                    kcol = (j % 4) * 256
                    psk = pps.tile([64, 512], f32, tag="ps")
                    for c in range(DC):
                        nc.tensor.matmul(psk, wk_sb[:, c, :],
                                         kt[:, c, kcol:kcol + 512],
                                         start=(c == 0), stop=(c == DC - 1))
                    nc.vector.tensor_copy(khT[:, 256 * j:256 * j + 512], psk)

                    psv = pps.tile([64, 512], f32, tag="ps")
                    for c in range(DC):
                        nc.tensor.matmul(psv, wv_sb[:, c, :],
                                         vt[:, c, kcol:kcol + 512],
                                         start=(c == 0), stop=(c == DC - 1))
                    vtmp = work.tile([64, 512], bf16, tag="vtmp")
                    nc.vector.tensor_copy(vtmp, psv)
                    last_vtmp = vtmp

                # v transposes for this group's two chunks; placed between
                # q proj and attention so PE covers the qh cast latency
                ptr = ptrps.tile([128, 128], bf16, tag="ptr")
                off = (j % 2) * 256
                for s in range(2):
                    nc.tensor.transpose(
                        ptr[:, 64 * s:64 * s + 64],
                        last_vtmp[:, off + 128 * s:off + 128 * s + 128],
                        identb[:64, :64])
                nc.vector.tensor_copy(
                    vh1[:, 2 * j:2 * j + 2, 0:H],
                    ptr.rearrange("p (a b) -> p a b", a=2))

                # ---- attention over owned kv chunks for this group ----
                nkv = counts[j]
                pvt = pvtps.tile([H + 1, 512], f32, tag="pvt")

                def pv_acc(m, psb, c0):
                    nc.tensor.matmul(pvt[:, c0:], vh1[:, m, :], psb[:, c0:],
                                     start=(m == 0), stop=(m == nkv - 1),
                                     skip_group_check=True)

                pend = None  # PV lags scores by one chunk so each chunk's
                for m in range(nkv):  # exp hides under the next score matmul
                    # last diagonal chunk only sees q columns >= 256 even in
                    # the worse parity; trim its score/exp/PV to that range
                    # (m==0 stays full so PSUM start=True covers all columns)
                    c0 = 256 if (apply_mask and m == nkv - 1) else 0
                    stp = stps.tile([128, 512], f32, tag="st")
                    nc.tensor.matmul(stp[:, c0:],
                                     khT[:, 128 * m:128 * m + 128],
                                     qh[:, c0:], start=True, stop=True)
                    psb = work.tile([128, 512], bf16, tag="p")
                    nc.scalar.activation(psb[:, c0:], stp[:, c0:],
                                         mybir.ActivationFunctionType.Exp,
                                         scale=scale)
                    if apply_mask and m == nkv - 2:
                        nc.vector.tensor_mul(psb, psb, dm_sb[:, 0, :])
                    if apply_mask and m == nkv - 1:
                        nc.vector.tensor_mul(psb[:, c0:], psb[:, c0:],
                                             dm_sb[:, 1, c0:])
                    if pend is not None:
                        pv_acc(*pend)
                    pend = (m, psb, c0)
                pv_acc(*pend)
                po = work.tile([H + 1, 512], f32, tag="po")
                nc.vector.tensor_copy(po, pvt)
                nc.sync.dma_start(out=out[:, 512 * j:512 * j + 512], in_=po)
    nc.compile()
    return nc


def _get_program(key, counts, apply_mask):
    if key not in _CACHE:
        _CACHE[key] = _build_program(counts, apply_mask)
    return _CACHE[key]


def _numpy_fallback(q, k, v, mask, Wq, Wk, Wv):
    qh = q.astype(np.float32) @ Wq
    kh = k.astype(np.float32) @ Wk
    vh = v.astype(np.float32) @ Wv
    out = np.empty((B, T, H), np.float32)
    neg = np.float32(-1e30)
    for b in range(B):
        s = (qh[b] @ kh[b].T) / np.float32(np.sqrt(H))
        s = np.where(mask == 0, neg, s)
        s = s - s.max(axis=-1, keepdims=True)
        e = np.exp(s)
        w = e / e.sum(axis=-1, keepdims=True)
        out[b] = w @ vh[b]
    return out


def _pmajor(x):
    """[D, N] -> [128, D//128, N] with d-low on partitions."""
    d, n = x.shape
    return np.ascontiguousarray(
        x.reshape(DC, 128, n).transpose(1, 0, 2))


def _make_in_maps(q, k, v, mask, Wq, Wk, Wv, apply_mask, np_in):
    in_maps = []
    for c in range(8):
        b, h = divmod(c, 2)
        qTp = _pmajor(np.ascontiguousarray(q[b].T)).astype(np_in)
        ko = np.ascontiguousarray(
            k[b].reshape(T // 128, 128, D)[h::2]
            .transpose(2, 0, 1).reshape(D, NO * 128))
        vo = np.ascontiguousarray(
            v[b].reshape(T // 128, 128, D)[h::2]
            .transpose(2, 0, 1).reshape(D, NO * 128))
        im = {
            "qT": qTp,
            "kT": _pmajor(ko).astype(np_in),
            "vT": _pmajor(vo).astype(np_in),
            "wq": _pmajor(Wq).astype(np_in),
            "wk": _pmajor(Wk).astype(np_in),
            "wv": _pmajor(Wv).astype(np_in),
        }
        if apply_mask:
            p = np.arange(128)[:, None]
            cc = np.arange(512)[None, :]
            dm = np.zeros((128, 2, 512), np.float32)
            dm[:, 0, :] = (cc >= p + 128 * h)
            dm[:, 1, :] = (cc >= p + 256 + 128 * h)
            im["dmask"] = dm.astype(np_in)
        in_maps.append(im)
    return in_maps


def _combine(results):
    out = np.empty((B, T, H), np.float32)
    for b in range(B):
        s = results[2 * b]["out"] + results[2 * b + 1]["out"]
        out[b] = (s[:H] / s[H:H + 1]).T
    return out


def kernel(q, k, v, mask, Wq, Wk, Wv):
    from concourse.bass_utils import run_bass_kernel_spmd
    import ml_dtypes

    q = np.ascontiguousarray(q, np.float32)
    k = np.ascontiguousarray(k, np.float32)
    v = np.ascontiguousarray(v, np.float32)
    Wq = np.ascontiguousarray(Wq, np.float32)
    Wk = np.ascontiguousarray(Wk, np.float32)
    Wv = np.ascontiguousarray(Wv, np.float32)
    mask = np.asarray(mask)

    # fast path is causal-only: its group loop projects KV chunks just
    # ahead of their first (causal) use, which would deadlock the in-order
    # PE queue for masks that attend ahead of the diagonal
    is_tril = bool((mask == np.tril(np.ones((T, T), mask.dtype))).all())
    if not is_tril:
        return _numpy_fallback(q, k, v, mask, Wq, Wk, Wv)

    np_in = ml_dtypes.bfloat16
    counts = [2 * j + 2 for j in range(NG)]
    nc = _get_program(("v9", True), counts, True)

    in_maps = _make_in_maps(q, k, v, mask, Wq, Wk, Wv, True, np_in)
    res = run_bass_kernel_spmd(nc, in_maps, list(range(8)))
    return _combine(res.results)


# revision 31
# speedup vs baseline: 1.0325x; 1.0325x over previous
"""Trainium2 Bass kernel for single-head causal attention with projections.

Reference computation (B=4, T=4096, D=1024, H=64):
    qh = q @ Wq; kh = k @ Wk; vh = v @ Wv          # [B,T,H]
    S  = qh @ kh.T / sqrt(H)  (causal masked)       # [B,T,T]
    out = softmax(S) @ vh                           # [B,T,H]

Sharding: 8 cores = 4 batches x 2 KV-parity halves. Each core owns its
batch's FULL q rows and the alternating 128-wide KV chunks of one parity,
so causal work balances exactly and no projection work is duplicated
within a core pair (q proj is duplicated instead of k+v, which is
cheaper). Cores return unnormalized partial attention accumulators
PV^T [H+1, T] (ones-column = exp-sum denominators); the host adds the
two parity halves per batch and normalizes - removing all on-device
transposes/reciprocals at the kernel tail and keeping a single big
output DMA.

All matmuls run in bf16 (moving-operand cost 1 cycle/column; fp32r was
1.24x slower on HW and fp8 fails the accuracy budget). Scores compute in
"ST orientation" (kv on partitions, q free) so exp(S^T) feeds the PV
matmul directly. No running max: scores are O(5) for this data regime.
Diagonal-chunk causal masks are two constant [128, 512] patterns
(group-invariant), multiplied in after exp.
"""

import numpy as np

B, T, D, H = 4, 4096, 1024, 64
DC = D // 128        # d chunks
NG = T // 512        # q groups of 512 rows
NO = T // 256        # owned kv chunks per core (16 of 32, alternating)

_CACHE = {}


def _build_program(counts, apply_mask):
    import concourse.bacc as bacc
    import concourse.mybir as mybir
    import concourse.tile as tile
    from concourse.masks import make_identity

    f32 = mybir.dt.float32
    bf16 = mybir.dt.bfloat16

    nc = bacc.Bacc(None, target_bir_lowering=False, debug=False)
    qT = nc.declare_dram_parameter("qT", [128, DC, T], bf16, isOutput=False)
    kT = nc.declare_dram_parameter("kT", [128, DC, NO * 128], bf16,
                                   isOutput=False)
    vT = nc.declare_dram_parameter("vT", [128, DC, NO * 128], bf16,
                                   isOutput=False)
    wq = nc.declare_dram_parameter("wq", [128, DC, H], bf16, isOutput=False)
    wk = nc.declare_dram_parameter("wk", [128, DC, H], bf16, isOutput=False)
    wv = nc.declare_dram_parameter("wv", [128, DC, H], bf16, isOutput=False)
    if apply_mask:
        dmask = nc.declare_dram_parameter("dmask", [128, 2, 512], bf16,
                                          isOutput=False)
    out = nc.declare_dram_parameter("out", [H + 1, T], f32, isOutput=True)

    scale = 1.0 / float(np.sqrt(H))

    with tile.TileContext(nc) as tc:
        with (
            tc.tile_pool(name="singles", bufs=1) as singles,
            tc.tile_pool(name="qstream", bufs=3) as qstream,
            tc.tile_pool(name="kvstream", bufs=2) as kvstream,
            tc.tile_pool(name="work", bufs=4) as work,
            tc.tile_pool(name="proj_ps", bufs=3, space="PSUM") as pps,
            tc.tile_pool(name="st_ps", bufs=2, space="PSUM") as stps,
            tc.tile_pool(name="ptr_ps", bufs=1, space="PSUM") as ptrps,
            tc.tile_pool(name="pvt_ps", bufs=1, space="PSUM") as pvtps,
        ):
            wq_sb = singles.tile([128, DC, H], bf16, tag="wq")
            wk_sb = singles.tile([128, DC, H], bf16, tag="wk")
            wv_sb = singles.tile([128, DC, H], bf16, tag="wv")
            nc.sync.dma_start(out=wq_sb, in_=wq[:, :, :])

            khT = singles.tile([64, NO * 128], bf16, tag="khT")
            vh1 = singles.tile([128, NO, H + 1], bf16, tag="vh1")

            # ---- startup: spread DMA issues over sync/scalar/gpsimd so
            # issue serialization (~650ns each) doesn't gate group 0; 1KB
            # line strips self-limit DMA rate, which keeps the HAM power
            # throttle mostly away (2KB max-rate bursts trip it) ----
            qt0 = qstream.tile([128, DC, 1024], bf16, tag="qt")
            for c in range(0, DC, 2):  # group-0 q, consumption-ordered
                nc.sync.dma_start(out=qt0[:, c:c + 2, 0:512],
                                  in_=qT[:, c:c + 2, 0:512])
            kt0 = kvstream.tile([128, DC, 1024], bf16, tag="kt")
            vt0 = kvstream.tile([128, DC, 1024], bf16, tag="vt")
            for c in range(0, DC, 2):  # groups 0-1 k then v, on scalar queue
                nc.scalar.dma_start(out=kt0[:, c:c + 2, 0:512],
                                    in_=kT[:, c:c + 2, 0:512])
            for c in range(0, DC, 2):
                nc.scalar.dma_start(out=vt0[:, c:c + 2, 0:512],
                                    in_=vT[:, c:c + 2, 0:512])
            # small singles on gpsimd so they don't occupy the sync queue
            nc.gpsimd.dma_start(out=wk_sb, in_=wk[:, :, :])
            nc.gpsimd.dma_start(out=wv_sb, in_=wv[:, :, :])
            if apply_mask:
                dm_sb = singles.tile([128, 2, 512], bf16, tag="dm")
                nc.gpsimd.dma_start(out=dm_sb, in_=dmask[:, :, :])

            # remaining input bulk: all on the sync HW-DGE queue, issued
            # back-to-back in consumption order; issue ops block on ring
            # backpressure so sync becomes a pipelined feeder that stays
            # ahead of the PE
            nc.sync.dma_start(out=qt0[:, :, 512:1024],
                              in_=qT[:, :, 512:1024])
            nc.sync.dma_start(out=kt0[:, :, 512:1024],
                              in_=kT[:, :, 512:1024])
            nc.sync.dma_start(out=vt0[:, :, 512:1024],
                              in_=vT[:, :, 512:1024])
            qts = [qt0]
            kts = {0: kt0}
            vts = {0: vt0}
            for g in (1,):
                qt = qstream.tile([128, DC, 1024], bf16, tag="qt")
                qts.append(qt)
                nc.sync.dma_start(
                    out=qt, in_=qT[:, :, 1024 * g:1024 * g + 1024])
            kt1 = kvstream.tile([128, DC, 1024], bf16, tag="kt")
            vt1 = kvstream.tile([128, DC, 1024], bf16, tag="vt")
            kts[1], vts[1] = kt1, vt1
            nc.sync.dma_start(out=kt1, in_=kT[:, :, 1024:2048])
            nc.sync.dma_start(out=vt1, in_=vT[:, :, 1024:2048])
            for g in (2, 3):
                qt = qstream.tile([128, DC, 1024], bf16, tag="qt")
                qts.append(qt)
                nc.sync.dma_start(
                    out=qt, in_=qT[:, :, 1024 * g:1024 * g + 1024])

            identf = singles.tile([128, 128], f32, tag="identf")
            make_identity(nc, identf)
            identb = singles.tile([128, 128], bf16, tag="identb")
            nc.vector.tensor_copy(identb, identf)
            nc.vector.memset(vh1[:, :, H:H + 1], 1.0)

            for j in range(NG):
                qt = qts[j // 2]
                qcol = (j % 2) * 512

                # ---- q projection for this group ----
                psq = pps.tile([64, 512], f32, tag="ps")
                for c in range(DC):
                    nc.tensor.matmul(psq, wq_sb[:, c, :],
                                     qt[:, c, qcol:qcol + 512],
                                     start=(c == 0), stop=(c == DC - 1))
                qh = work.tile([64, 512], bf16, tag="qh")
                nc.vector.tensor_copy(qh, psq)

                # ---- k/v projections, two groups at a time (512 cols) ----
                if j % 2 == 0:
                    kt, vt = kts[j // 4], vts[j // 4]
                    kcol = (j % 4) * 256
                    psk = pps.tile([64, 512], f32, tag="ps")
                    for c in range(DC):
                        nc.tensor.matmul(psk, wk_sb[:, c, :],
                                         kt[:, c, kcol:kcol + 512],
                                         start=(c == 0), stop=(c == DC - 1))
                    nc.vector.tensor_copy(khT[:, 256 * j:256 * j + 512], psk)

                    psv = pps.tile([64, 512], f32, tag="ps")
                    for c in range(DC):
                        nc.tensor.matmul(psv, wv_sb[:, c, :],
                                         vt[:, c, kcol:kcol + 512],
                                         start=(c == 0), stop=(c == DC - 1))
                    vtmp = work.tile([64, 512], bf16, tag="vtmp")
                    nc.vector.tensor_copy(vtmp, psv)
                    last_vtmp = vtmp

                # v transposes for this group's two chunks; placed between
                # q proj and attention so PE covers the qh cast latency
                ptr = ptrps.tile([128, 128], bf16, tag="ptr")
                off = (j % 2) * 256
                for s in range(2):
                    nc.tensor.transpose(
                        ptr[:, 64 * s:64 * s + 64],
                        last_vtmp[:, off + 128 * s:off + 128 * s + 128],
                        identb[:64, :64])
                nc.vector.tensor_copy(
                    vh1[:, 2 * j:2 * j + 2, 0:H],
                    ptr.rearrange("p (a b) -> p a b", a=2))

                # ---- attention over owned kv chunks for this group ----
                nkv = counts[j]
                pvt = pvtps.tile([H + 1, 512], f32, tag="pvt")
                for m in range(nkv):
                    # last diagonal chunk only sees q columns >= 256 even in
                    # the worse parity; trim its score/exp/PV to that range
                    # (m==0 stays full so PSUM start=True covers all columns)
                    c0 = 256 if (apply_mask and m == nkv - 1) else 0
                    stp = stps.tile([128, 512], f32, tag="st")
                    nc.tensor.matmul(stp[:, c0:],
                                     khT[:, 128 * m:128 * m + 128],
                                     qh[:, c0:], start=True, stop=True)
                    psb = work.tile([128, 512], bf16, tag="p")
                    nc.scalar.activation(psb[:, c0:], stp[:, c0:],
                                         mybir.ActivationFunctionType.Exp,
                                         scale=scale)
                    if apply_mask and m == nkv - 2:
                        nc.vector.tensor_mul(psb, psb, dm_sb[:, 0, :])
                    if apply_mask and m == nkv - 1:
                        nc.vector.tensor_mul(psb[:, c0:], psb[:, c0:],
                                             dm_sb[:, 1, c0:])
                    nc.tensor.matmul(pvt[:, c0:], vh1[:, m, :], psb[:, c0:],
                                     start=(m == 0), stop=(m == nkv - 1),
                                     skip_group_check=True)
                po = work.tile([H + 1, 512], f32, tag="po")
                nc.vector.tensor_copy(po, pvt)
                nc.sync.dma_start(out=out[:, 512 * j:512 * j + 512], in_=po)
    nc.compile()
    return nc


def _get_program(key, counts, apply_mask):
    if key not in _CACHE:
        _CACHE[key] = _build_program(counts, apply_mask)
    return _CACHE[key]


def _numpy_fallback(q, k, v, mask, Wq, Wk, Wv):
    qh = q.astype(np.float32) @ Wq
    kh = k.astype(np.float32) @ Wk
    vh = v.astype(np.float32) @ Wv
    out = np.empty((B, T, H), np.float32)
    neg = np.float32(-1e30)
    for b in range(B):
        s = (qh[b] @ kh[b].T) / np.float32(np.sqrt(H))
        s = np.where(mask == 0, neg, s)
        s = s - s.max(axis=-1, keepdims=True)
        e = np.exp(s)
        w = e / e.sum(axis=-1, keepdims=True)
        out[b] = w @ vh[b]
    return out


def _pmajor(x):
    """[D, N] -> [128, D//128, N] with d-low on partitions."""
    d, n = x.shape
    return np.ascontiguousarray(
        x.reshape(DC, 128, n).transpose(1, 0, 2))


def _make_in_maps(q, k, v, mask, Wq, Wk, Wv, apply_mask, np_in):
    in_maps = []
    for c in range(8):
        b, h = divmod(c, 2)
        qTp = _pmajor(np.ascontiguousarray(q[b].T)).astype(np_in)
        ko = np.ascontiguousarray(
            k[b].reshape(T // 128, 128, D)[h::2]
            .transpose(2, 0, 1).reshape(D, NO * 128))
        vo = np.ascontiguousarray(
            v[b].reshape(T // 128, 128, D)[h::2]
            .transpose(2, 0, 1).reshape(D, NO * 128))
        im = {
            "qT": qTp,
            "kT": _pmajor(ko).astype(np_in),
            "vT": _pmajor(vo).astype(np_in),
            "wq": _pmajor(Wq).astype(np_in),
            "wk": _pmajor(Wk).astype(np_in),
            "wv": _pmajor(Wv).astype(np_in),
        }
        if apply_mask:
            p = np.arange(128)[:, None]
            cc = np.arange(512)[None, :]
            dm = np.zeros((128, 2, 512), np.float32)
            dm[:, 0, :] = (cc >= p + 128 * h)
            dm[:, 1, :] = (cc >= p + 256 + 128 * h)
            im["dmask"] = dm.astype(np_in)
        in_maps.append(im)
    return in_maps


def _combine(results):
    out = np.empty((B, T, H), np.float32)
    for b in range(B):
        s = results[2 * b]["out"] + results[2 * b + 1]["out"]
        out[b] = (s[:H] / s[H:H + 1]).T
    return out


def kernel(q, k, v, mask, Wq, Wk, Wv):
    from concourse.bass_utils import run_bass_kernel_spmd
    import ml_dtypes

    q = np.ascontiguousarray(q, np.float32)
    k = np.ascontiguousarray(k, np.float32)
    v = np.ascontiguousarray(v, np.float32)
    Wq = np.ascontiguousarray(Wq, np.float32)
    Wk = np.ascontiguousarray(Wk, np.float32)
    Wv = np.ascontiguousarray(Wv, np.float32)
    mask = np.asarray(mask)

    # fast path is causal-only: its group loop projects KV chunks just
    # ahead of their first (causal) use, which would deadlock the in-order
    # PE queue for masks that attend ahead of the diagonal
    is_tril = bool((mask == np.tril(np.ones((T, T), mask.dtype))).all())
    if not is_tril:
        return _numpy_fallback(q, k, v, mask, Wq, Wk, Wv)

    np_in = ml_dtypes.bfloat16
    counts = [2 * j + 2 for j in range(NG)]
    nc = _get_program(("v9", True), counts, True)

    in_maps = _make_in_maps(q, k, v, mask, Wq, Wk, Wv, True, np_in)
    res = run_bass_kernel_spmd(nc, in_maps, list(range(8)))
    return _combine(res.results)


# revision 35
# speedup vs baseline: 1.0344x; 1.0019x over previous
"""Trainium2 Bass kernel for single-head causal attention with projections.

Reference computation (B=4, T=4096, D=1024, H=64):
    qh = q @ Wq; kh = k @ Wk; vh = v @ Wv          # [B,T,H]
    S  = qh @ kh.T / sqrt(H)  (causal masked)       # [B,T,T]
    out = softmax(S) @ vh                           # [B,T,H]

Sharding: 8 cores = 4 batches x 2 KV-parity halves. Each core owns its
batch's FULL q rows and the alternating 128-wide KV chunks of one parity,
so causal work balances exactly and no projection work is duplicated
within a core pair (q proj is duplicated instead of k+v, which is
cheaper). Cores return unnormalized partial attention accumulators
PV^T [H+1, T] (ones-column = exp-sum denominators); the host adds the
two parity halves per batch and normalizes - removing all on-device
transposes/reciprocals at the kernel tail and keeping a single big
output DMA.

All matmuls run in bf16 (moving-operand cost 1 cycle/column; fp32r was
1.24x slower on HW and fp8 fails the accuracy budget). Scores compute in
"ST orientation" (kv on partitions, q free) so exp(S^T) feeds the PV
matmul directly. No running max: scores are O(5) for this data regime.
Diagonal-chunk causal masks are two constant [128, 512] patterns
(group-invariant), multiplied in after exp.
"""

import numpy as np

B, T, D, H = 4, 4096, 1024, 64
DC = D // 128        # d chunks
NG = T // 512        # q groups of 512 rows
NO = T // 256        # owned kv chunks per core (16 of 32, alternating)

_CACHE = {}


def _build_program(counts, apply_mask):
    import concourse.bacc as bacc
    import concourse.mybir as mybir
    import concourse.tile as tile
    from concourse.masks import make_identity

    f32 = mybir.dt.float32
    bf16 = mybir.dt.bfloat16

    nc = bacc.Bacc(None, target_bir_lowering=False, debug=False)
    qT = nc.declare_dram_parameter("qT", [128, DC, T], bf16, isOutput=False)
    kT = nc.declare_dram_parameter("kT", [128, DC, NO * 128], bf16,
                                   isOutput=False)
    vT = nc.declare_dram_parameter("vT", [128, DC, NO * 128], bf16,
                                   isOutput=False)
    # first-group slices host-repacked contiguous so startup strips get
    # 2KB DMA lines (double rate) instead of 1KB
    qT0 = nc.declare_dram_parameter("qT0", [128, DC, 512], bf16,
                                    isOutput=False)
    kT0 = nc.declare_dram_parameter("kT0", [128, DC, 512], bf16,
                                    isOutput=False)
    vT0 = nc.declare_dram_parameter("vT0", [128, DC, 512], bf16,
                                    isOutput=False)
    wq = nc.declare_dram_parameter("wq", [128, DC, H], bf16, isOutput=False)
    wk = nc.declare_dram_parameter("wk", [128, DC, H], bf16, isOutput=False)
    wv = nc.declare_dram_parameter("wv", [128, DC, H], bf16, isOutput=False)
    if apply_mask:
        dmask = nc.declare_dram_parameter("dmask", [128, 2, 512], bf16,
                                          isOutput=False)
    out = nc.declare_dram_parameter("out", [H + 1, T], f32, isOutput=True)

    scale = 1.0 / float(np.sqrt(H))

    with tile.TileContext(nc) as tc:
        with (
            tc.tile_pool(name="singles", bufs=1) as singles,
            tc.tile_pool(name="qstream", bufs=3) as qstream,
            tc.tile_pool(name="kvstream", bufs=2) as kvstream,
            tc.tile_pool(name="work", bufs=4) as work,
            tc.tile_pool(name="proj_ps", bufs=3, space="PSUM") as pps,
            tc.tile_pool(name="st_ps", bufs=2, space="PSUM") as stps,
            tc.tile_pool(name="ptr_ps", bufs=1, space="PSUM") as ptrps,
            tc.tile_pool(name="pvt_ps", bufs=1, space="PSUM") as pvtps,
        ):
            wq_sb = singles.tile([128, DC, H], bf16, tag="wq")
            wk_sb = singles.tile([128, DC, H], bf16, tag="wk")
            wv_sb = singles.tile([128, DC, H], bf16, tag="wv")
            nc.sync.dma_start(out=wq_sb, in_=wq[:, :, :])

            khT = singles.tile([64, NO * 128], bf16, tag="khT")
            vh1 = singles.tile([128, NO, H + 1], bf16, tag="vh1")

            # ---- startup: spread DMA issues over sync/scalar/gpsimd so
            # issue serialization (~650ns each) doesn't gate group 0; 1KB
            # line strips self-limit DMA rate, which keeps the HAM power
            # throttle mostly away (2KB max-rate bursts trip it) ----
            qt0 = qstream.tile([128, DC, 1024], bf16, tag="qt")
            for c in range(0, DC, 2):  # group-0 q, consumption-ordered
                nc.sync.dma_start(out=qt0[:, c:c + 2, 0:512],
                                  in_=qT0[:, c:c + 2, :])
            kt0 = kvstream.tile([128, DC, 1024], bf16, tag="kt")
            vt0 = kvstream.tile([128, DC, 1024], bf16, tag="vt")
            for c in range(0, DC, 2):  # groups 0-1 k then v, on scalar queue
                nc.scalar.dma_start(out=kt0[:, c:c + 2, 0:512],
                                    in_=kT0[:, c:c + 2, :])
            for c in range(0, DC, 2):
                nc.scalar.dma_start(out=vt0[:, c:c + 2, 0:512],
                                    in_=vT0[:, c:c + 2, :])
            # small singles on gpsimd so they don't occupy the sync queue
            nc.gpsimd.dma_start(out=wk_sb, in_=wk[:, :, :])
            nc.gpsimd.dma_start(out=wv_sb, in_=wv[:, :, :])
            if apply_mask:
                dm_sb = singles.tile([128, 2, 512], bf16, tag="dm")
                nc.gpsimd.dma_start(out=dm_sb, in_=dmask[:, :, :])

            # remaining input bulk: all on the sync HW-DGE queue, issued
            # back-to-back in consumption order; issue ops block on ring
            # backpressure so sync becomes a pipelined feeder that stays
            # ahead of the PE
            nc.sync.dma_start(out=qt0[:, :, 512:1024],
                              in_=qT[:, :, 512:1024])
            nc.sync.dma_start(out=kt0[:, :, 512:1024],
                              in_=kT[:, :, 512:1024])
            nc.sync.dma_start(out=vt0[:, :, 512:1024],
                              in_=vT[:, :, 512:1024])
            qts = [qt0]
            kts = {0: kt0}
            vts = {0: vt0}
            for g in (1,):
                qt = qstream.tile([128, DC, 1024], bf16, tag="qt")
                qts.append(qt)
                nc.sync.dma_start(
                    out=qt, in_=qT[:, :, 1024 * g:1024 * g + 1024])
            kt1 = kvstream.tile([128, DC, 1024], bf16, tag="kt")
            vt1 = kvstream.tile([128, DC, 1024], bf16, tag="vt")
            kts[1], vts[1] = kt1, vt1
            nc.sync.dma_start(out=kt1, in_=kT[:, :, 1024:2048])
            nc.sync.dma_start(out=vt1, in_=vT[:, :, 1024:2048])
            for g in (2, 3):
                qt = qstream.tile([128, DC, 1024], bf16, tag="qt")
                qts.append(qt)
                nc.sync.dma_start(
                    out=qt, in_=qT[:, :, 1024 * g:1024 * g + 1024])

            identf = singles.tile([128, 128], f32, tag="identf")
            make_identity(nc, identf)
            identb = singles.tile([128, 128], bf16, tag="identb")
            nc.vector.tensor_copy(identb, identf)
            nc.vector.memset(vh1[:, :, H:H + 1], 1.0)

            for j in range(NG):
                qt = qts[j // 2]
                qcol = (j % 2) * 512

                # ---- q projection for this group ----
                psq = pps.tile([64, 512], f32, tag="ps")
                for c in range(DC):
                    nc.tensor.matmul(psq, wq_sb[:, c, :],
                                     qt[:, c, qcol:qcol + 512],
                                     start=(c == 0), stop=(c == DC - 1))
                qh = work.tile([64, 512], bf16, tag="qh")
                nc.vector.tensor_copy(qh, psq)

                # ---- k/v projections, two groups at a time (512 cols) ----
                if j % 2 == 0:
                    kt, vt = kts[j // 4], vts[j // 4]
                    kcol = (j % 4) * 256
                    psk = pps.tile([64, 512], f32, tag="ps")
                    for c in range(DC):
                        nc.tensor.matmul(psk, wk_sb[:, c, :],
                                         kt[:, c, kcol:kcol + 512],
                                         start=(c == 0), stop=(c == DC - 1))
                    nc.vector.tensor_copy(khT[:, 256 * j:256 * j + 512], psk)

                    psv = pps.tile([64, 512], f32, tag="ps")
                    for c in range(DC):
                        nc.tensor.matmul(psv, wv_sb[:, c, :],
                                         vt[:, c, kcol:kcol + 512],
                                         start=(c == 0), stop=(c == DC - 1))
                    vtmp = work.tile([64, 512], bf16, tag="vtmp")
                    nc.vector.tensor_copy(vtmp, psv)
                    last_vtmp = vtmp

                # v transposes for this group's two chunks; placed between
                # q proj and attention so PE covers the qh cast latency
                ptr = ptrps.tile([128, 128], bf16, tag="ptr")
                off = (j % 2) * 256
                for s in range(2):
                    nc.tensor.transpose(
                        ptr[:, 64 * s:64 * s + 64],
                        last_vtmp[:, off + 128 * s:off + 128 * s + 128],
                        identb[:64, :64])
                nc.vector.tensor_copy(
                    vh1[:, 2 * j:2 * j + 2, 0:H],
                    ptr.rearrange("p (a b) -> p a b", a=2))

                # ---- attention over owned kv chunks for this group ----
                nkv = counts[j]
                pvt = pvtps.tile([H + 1, 512], f32, tag="pvt")
                for m in range(nkv):
                    # last diagonal chunk only sees q columns >= 256 even in
                    # the worse parity; trim its score/exp/PV to that range
                    # (m==0 stays full so PSUM start=True covers all columns)
                    c0 = 256 if (apply_mask and m == nkv - 1) else 0
                    stp = stps.tile([128, 512], f32, tag="st")
                    nc.tensor.matmul(stp[:, c0:],
                                     khT[:, 128 * m:128 * m + 128],
                                     qh[:, c0:], start=True, stop=True)
                    psb = work.tile([128, 512], bf16, tag="p")
                    nc.scalar.activation(psb[:, c0:], stp[:, c0:],
                                         mybir.ActivationFunctionType.Exp,
                                         scale=scale)
                    if apply_mask and m == nkv - 2:
                        nc.vector.tensor_mul(psb, psb, dm_sb[:, 0, :])
                    if apply_mask and m == nkv - 1:
                        nc.vector.tensor_mul(psb[:, c0:], psb[:, c0:],
                                             dm_sb[:, 1, c0:])
                    nc.tensor.matmul(pvt[:, c0:], vh1[:, m, :], psb[:, c0:],
                                     start=(m == 0), stop=(m == nkv - 1),
                                     skip_group_check=True)
                po = work.tile([H + 1, 512], f32, tag="po")
                nc.vector.tensor_copy(po, pvt)
                nc.sync.dma_start(out=out[:, 512 * j:512 * j + 512], in_=po)
    nc.compile()
    return nc


def _get_program(key, counts, apply_mask):
    if key not in _CACHE:
        _CACHE[key] = _build_program(counts, apply_mask)
    return _CACHE[key]


def _numpy_fallback(q, k, v, mask, Wq, Wk, Wv):
    qh = q.astype(np.float32) @ Wq
    kh = k.astype(np.float32) @ Wk
    vh = v.astype(np.float32) @ Wv
    out = np.empty((B, T, H), np.float32)
    neg = np.float32(-1e30)
    for b in range(B):
        s = (qh[b] @ kh[b].T) / np.float32(np.sqrt(H))
        s = np.where(mask == 0, neg, s)
        s = s - s.max(axis=-1, keepdims=True)
        e = np.exp(s)
        w = e / e.sum(axis=-1, keepdims=True)
        out[b] = w @ vh[b]
    return out


def _pmajor(x):
    """[D, N] -> [128, D//128, N] with d-low on partitions."""
    d, n = x.shape
    return np.ascontiguousarray(
        x.reshape(DC, 128, n).transpose(1, 0, 2))


def _make_in_maps(q, k, v, mask, Wq, Wk, Wv, apply_mask, np_in):
    in_maps = []
    for c in range(8):
        b, h = divmod(c, 2)
        qTp = _pmajor(np.ascontiguousarray(q[b].T)).astype(np_in)
        ko = np.ascontiguousarray(
            k[b].reshape(T // 128, 128, D)[h::2]
            .transpose(2, 0, 1).reshape(D, NO * 128))
        vo = np.ascontiguousarray(
            v[b].reshape(T // 128, 128, D)[h::2]
            .transpose(2, 0, 1).reshape(D, NO * 128))
        kTp = _pmajor(ko).astype(np_in)
        vTp = _pmajor(vo).astype(np_in)
        im = {
            "qT": qTp,
            "kT": kTp,
            "vT": vTp,
            "qT0": np.ascontiguousarray(qTp[:, :, 0:512]),
            "kT0": np.ascontiguousarray(kTp[:, :, 0:512]),
            "vT0": np.ascontiguousarray(vTp[:, :, 0:512]),
            "wq": _pmajor(Wq).astype(np_in),
            "wk": _pmajor(Wk).astype(np_in),
            "wv": _pmajor(Wv).astype(np_in),
        }
        if apply_mask:
            p = np.arange(128)[:, None]
            cc = np.arange(512)[None, :]
            dm = np.zeros((128, 2, 512), np.float32)
            dm[:, 0, :] = (cc >= p + 128 * h)
            dm[:, 1, :] = (cc >= p + 256 + 128 * h)
            im["dmask"] = dm.astype(np_in)
        in_maps.append(im)
    return in_maps


def _combine(results):
    out = np.empty((B, T, H), np.float32)
    for b in range(B):
        s = results[2 * b]["out"] + results[2 * b + 1]["out"]
        out[b] = (s[:H] / s[H:H + 1]).T
    return out


def kernel(q, k, v, mask, Wq, Wk, Wv):
    from concourse.bass_utils import run_bass_kernel_spmd
    import ml_dtypes

    q = np.ascontiguousarray(q, np.float32)
    k = np.ascontiguousarray(k, np.float32)
    v = np.ascontiguousarray(v, np.float32)
    Wq = np.ascontiguousarray(Wq, np.float32)
    Wk = np.ascontiguousarray(Wk, np.float32)
    Wv = np.ascontiguousarray(Wv, np.float32)
    mask = np.asarray(mask)

    # fast path is causal-only: its group loop projects KV chunks just
    # ahead of their first (causal) use, which would deadlock the in-order
    # PE queue for masks that attend ahead of the diagonal
    is_tril = bool((mask == np.tril(np.ones((T, T), mask.dtype))).all())
    if not is_tril:
        return _numpy_fallback(q, k, v, mask, Wq, Wk, Wv)

    np_in = ml_dtypes.bfloat16
    counts = [2 * j + 2 for j in range(NG)]
    nc = _get_program(("v11", True), counts, True)

    in_maps = _make_in_maps(q, k, v, mask, Wq, Wk, Wv, True, np_in)
    res = run_bass_kernel_spmd(nc, in_maps, list(range(8)))
    return _combine(res.results)


# revision 37
# speedup vs baseline: 1.0458x; 1.0109x over previous
"""Trainium2 Bass kernel for single-head causal attention with projections.

Reference computation (B=4, T=4096, D=1024, H=64):
    qh = q @ Wq; kh = k @ Wk; vh = v @ Wv          # [B,T,H]
    S  = qh @ kh.T / sqrt(H)  (causal masked)       # [B,T,T]
    out = softmax(S) @ vh                           # [B,T,H]

Sharding: 8 cores = 4 batches x 2 KV-parity halves. Each core owns its
batch's FULL q rows and the alternating 128-wide KV chunks of one parity,
so causal work balances exactly and no projection work is duplicated
within a core pair (q proj is duplicated instead of k+v, which is
cheaper). Cores return unnormalized partial attention accumulators
PV^T [H+1, T] (ones-column = exp-sum denominators); the host adds the
two parity halves per batch and normalizes - removing all on-device
transposes/reciprocals at the kernel tail and keeping a single big
output DMA.

All matmuls run in bf16 (moving-operand cost 1 cycle/column; fp32r was
1.24x slower on HW and fp8 fails the accuracy budget). Scores compute in
"ST orientation" (kv on partitions, q free) so exp(S^T) feeds the PV
matmul directly. No running max: scores are O(5) for this data regime.
Diagonal-chunk causal masks are two constant [128, 512] patterns
(group-invariant), multiplied in after exp.
"""

import numpy as np

B, T, D, H = 4, 4096, 1024, 64
DC = D // 128        # d chunks
NG = T // 512        # q groups of 512 rows
NO = T // 256        # owned kv chunks per core (16 of 32, alternating)

_CACHE = {}


def _build_program(counts, apply_mask):
    import concourse.bacc as bacc
    import concourse.mybir as mybir
    import concourse.tile as tile
    from concourse.masks import make_identity

    f32 = mybir.dt.float32
    bf16 = mybir.dt.bfloat16

    nc = bacc.Bacc(None, target_bir_lowering=False, debug=False)
    qT = nc.declare_dram_parameter("qT", [128, DC, T], bf16, isOutput=False)
    kT = nc.declare_dram_parameter("kT", [128, DC, NO * 128], bf16,
                                   isOutput=False)
    vT = nc.declare_dram_parameter("vT", [128, DC, NO * 128], bf16,
                                   isOutput=False)
    # first-group slices host-repacked contiguous so startup strips get
    # 2KB DMA lines (double rate) instead of 1KB
    qT0 = nc.declare_dram_parameter("qT0", [128, DC, 512], bf16,
                                    isOutput=False)
    kT0 = nc.declare_dram_parameter("kT0", [128, DC, 512], bf16,
                                    isOutput=False)
    vT0 = nc.declare_dram_parameter("vT0", [128, DC, 512], bf16,
                                    isOutput=False)
    wq = nc.declare_dram_parameter("wq", [128, DC, H], bf16, isOutput=False)
    wk = nc.declare_dram_parameter("wk", [128, DC, H], bf16, isOutput=False)
    wv = nc.declare_dram_parameter("wv", [128, DC, H], bf16, isOutput=False)
    if apply_mask:
        dmask = nc.declare_dram_parameter("dmask", [128, 2, 512], bf16,
                                          isOutput=False)
    out = nc.declare_dram_parameter("out", [H + 1, T], f32, isOutput=True)

    scale = 1.0 / float(np.sqrt(H))

    with tile.TileContext(nc) as tc:
        with (
            tc.tile_pool(name="singles", bufs=1) as singles,
            tc.tile_pool(name="qstream", bufs=3) as qstream,
            tc.tile_pool(name="kvstream", bufs=2) as kvstream,
            tc.tile_pool(name="work", bufs=4) as work,
            tc.tile_pool(name="proj_ps", bufs=3, space="PSUM") as pps,
            tc.tile_pool(name="st_ps", bufs=3, space="PSUM") as stps,
            tc.tile_pool(name="ptr_ps", bufs=1, space="PSUM") as ptrps,
            tc.tile_pool(name="pvt_ps", bufs=1, space="PSUM") as pvtps,
        ):
            wq_sb = singles.tile([128, DC, H], bf16, tag="wq")
            wk_sb = singles.tile([128, DC, H], bf16, tag="wk")
            wv_sb = singles.tile([128, DC, H], bf16, tag="wv")
            nc.sync.dma_start(out=wq_sb, in_=wq[:, :, :])

            khT = singles.tile([64, NO * 128], bf16, tag="khT")
            vh1 = singles.tile([128, NO, H + 1], bf16, tag="vh1")

            # ---- startup: spread DMA issues over sync/scalar/gpsimd so
            # issue serialization (~650ns each) doesn't gate group 0; 1KB
            # line strips self-limit DMA rate, which keeps the HAM power
            # throttle mostly away (2KB max-rate bursts trip it) ----
            qt0 = qstream.tile([128, DC, 1024], bf16, tag="qt")
            for c in range(0, DC, 2):  # group-0 q, consumption-ordered
                nc.sync.dma_start(out=qt0[:, c:c + 2, 0:512],
                                  in_=qT0[:, c:c + 2, :])
            kt0 = kvstream.tile([128, DC, 1024], bf16, tag="kt")
            vt0 = kvstream.tile([128, DC, 1024], bf16, tag="vt")
            for c in range(0, DC, 2):  # groups 0-1 k then v, on scalar queue
                nc.scalar.dma_start(out=kt0[:, c:c + 2, 0:512],
                                    in_=kT0[:, c:c + 2, :])
            for c in range(0, DC, 2):
                nc.scalar.dma_start(out=vt0[:, c:c + 2, 0:512],
                                    in_=vT0[:, c:c + 2, :])
            # small singles on gpsimd so they don't occupy the sync queue
            nc.gpsimd.dma_start(out=wk_sb, in_=wk[:, :, :])
            nc.gpsimd.dma_start(out=wv_sb, in_=wv[:, :, :])
            if apply_mask:
                dm_sb = singles.tile([128, 2, 512], bf16, tag="dm")
                nc.gpsimd.dma_start(out=dm_sb, in_=dmask[:, :, :])

            # remaining input bulk: all on the sync HW-DGE queue, issued
            # back-to-back in consumption order; issue ops block on ring
            # backpressure so sync becomes a pipelined feeder that stays
            # ahead of the PE
            nc.sync.dma_start(out=qt0[:, :, 512:1024],
                              in_=qT[:, :, 512:1024])
            nc.sync.dma_start(out=kt0[:, :, 512:1024],
                              in_=kT[:, :, 512:1024])
            nc.sync.dma_start(out=vt0[:, :, 512:1024],
                              in_=vT[:, :, 512:1024])
            qts = [qt0]
            kts = {0: kt0}
            vts = {0: vt0}
            for g in (1,):
                qt = qstream.tile([128, DC, 1024], bf16, tag="qt")
                qts.append(qt)
                nc.sync.dma_start(
                    out=qt, in_=qT[:, :, 1024 * g:1024 * g + 1024])
            kt1 = kvstream.tile([128, DC, 1024], bf16, tag="kt")
            vt1 = kvstream.tile([128, DC, 1024], bf16, tag="vt")
            kts[1], vts[1] = kt1, vt1
            nc.sync.dma_start(out=kt1, in_=kT[:, :, 1024:2048])
            nc.sync.dma_start(out=vt1, in_=vT[:, :, 1024:2048])
            for g in (2, 3):
                qt = qstream.tile([128, DC, 1024], bf16, tag="qt")
                qts.append(qt)
                nc.sync.dma_start(
                    out=qt, in_=qT[:, :, 1024 * g:1024 * g + 1024])

            identf = singles.tile([128, 128], f32, tag="identf")
            make_identity(nc, identf)
            identb = singles.tile([128, 128], bf16, tag="identb")
            nc.vector.tensor_copy(identb, identf)
            nc.vector.memset(vh1[:, :, H:H + 1], 1.0)

            for j in range(NG):
                qt = qts[j // 2]
                qcol = (j % 2) * 512

                # ---- q projection for this group ----
                psq = pps.tile([64, 512], f32, tag="ps")
                for c in range(DC):
                    nc.tensor.matmul(psq, wq_sb[:, c, :],
                                     qt[:, c, qcol:qcol + 512],
                                     start=(c == 0), stop=(c == DC - 1))
                qh = work.tile([64, 512], bf16, tag="qh")
                nc.vector.tensor_copy(qh, psq)

                # ---- k/v projections, two groups at a time (512 cols) ----
                if j % 2 == 0:
                    kt, vt = kts[j // 4], vts[j // 4]
                    kcol = (j % 4) * 256
                    psk = pps.tile([64, 512], f32, tag="ps")
                    for c in range(DC):
                        nc.tensor.matmul(psk, wk_sb[:, c, :],
                                         kt[:, c, kcol:kcol + 512],
                                         start=(c == 0), stop=(c == DC - 1))
                    nc.vector.tensor_copy(khT[:, 256 * j:256 * j + 512], psk)

                    psv = pps.tile([64, 512], f32, tag="ps")
                    for c in range(DC):
                        nc.tensor.matmul(psv, wv_sb[:, c, :],
                                         vt[:, c, kcol:kcol + 512],
                                         start=(c == 0), stop=(c == DC - 1))
                    vtmp = work.tile([64, 512], bf16, tag="vtmp")
                    nc.vector.tensor_copy(vtmp, psv)
                    last_vtmp = vtmp

                # v transposes for this group's two chunks; placed between
                # q proj and attention so PE covers the qh cast latency
                ptr = ptrps.tile([128, 128], bf16, tag="ptr")
                off = (j % 2) * 256
                for s in range(2):
                    nc.tensor.transpose(
                        ptr[:, 64 * s:64 * s + 64],
                        last_vtmp[:, off + 128 * s:off + 128 * s + 128],
                        identb[:64, :64])
                nc.vector.tensor_copy(
                    vh1[:, 2 * j:2 * j + 2, 0:H],
                    ptr.rearrange("p (a b) -> p a b", a=2))

                # ---- attention over owned kv chunks for this group ----
                nkv = counts[j]
                pvt = pvtps.tile([H + 1, 512], f32, tag="pvt")
                for m in range(nkv):
                    # last diagonal chunk only sees q columns >= 256 even in
                    # the worse parity; trim its score/exp/PV to that range
                    # (m==0 stays full so PSUM start=True covers all columns)
                    c0 = 256 if (apply_mask and m == nkv - 1) else 0
                    stp = stps.tile([128, 512], f32, tag="st")
                    nc.tensor.matmul(stp[:, c0:],
                                     khT[:, 128 * m:128 * m + 128],
                                     qh[:, c0:], start=True, stop=True)
                    psb = work.tile([128, 512], bf16, tag="p")
                    nc.scalar.activation(psb[:, c0:], stp[:, c0:],
                                         mybir.ActivationFunctionType.Exp,
                                         scale=scale)
                    if apply_mask and m == nkv - 2:
                        nc.vector.tensor_mul(psb, psb, dm_sb[:, 0, :])
                    if apply_mask and m == nkv - 1:
                        nc.vector.tensor_mul(psb[:, c0:], psb[:, c0:],
                                             dm_sb[:, 1, c0:])
                    nc.tensor.matmul(pvt[:, c0:], vh1[:, m, :], psb[:, c0:],
                                     start=(m == 0), stop=(m == nkv - 1),
                                     skip_group_check=True)
                po = work.tile([H + 1, 512], f32, tag="po")
                nc.vector.tensor_copy(po, pvt)
                nc.sync.dma_start(out=out[:, 512 * j:512 * j + 512], in_=po)
    nc.compile()
    return nc


def _get_program(key, counts, apply_mask):
    if key not in _CACHE:
        _CACHE[key] = _build_program(counts, apply_mask)
    return _CACHE[key]


def _numpy_fallback(q, k, v, mask, Wq, Wk, Wv):
    qh = q.astype(np.float32) @ Wq
    kh = k.astype(np.float32) @ Wk
    vh = v.astype(np.float32) @ Wv
    out = np.empty((B, T, H), np.float32)
    neg = np.float32(-1e30)
    for b in range(B):
        s = (qh[b] @ kh[b].T) / np.float32(np.sqrt(H))
        s = np.where(mask == 0, neg, s)
        s = s - s.max(axis=-1, keepdims=True)
        e = np.exp(s)
        w = e / e.sum(axis=-1, keepdims=True)
        out[b] = w @ vh[b]
    return out


def _pmajor(x):
    """[D, N] -> [128, D//128, N] with d-low on partitions."""
    d, n = x.shape
    return np.ascontiguousarray(
        x.reshape(DC, 128, n).transpose(1, 0, 2))


def _make_in_maps(q, k, v, mask, Wq, Wk, Wv, apply_mask, np_in):
    in_maps = []
    for c in range(8):
        b, h = divmod(c, 2)
        qTp = _pmajor(np.ascontiguousarray(q[b].T)).astype(np_in)
        ko = np.ascontiguousarray(
            k[b].reshape(T // 128, 128, D)[h::2]
            .transpose(2, 0, 1).reshape(D, NO * 128))
        vo = np.ascontiguousarray(
            v[b].reshape(T // 128, 128, D)[h::2]
            .transpose(2, 0, 1).reshape(D, NO * 128))
        kTp = _pmajor(ko).astype(np_in)
        vTp = _pmajor(vo).astype(np_in)
        im = {
            "qT": qTp,
            "kT": kTp,
            "vT": vTp,
            "qT0": np.ascontiguousarray(qTp[:, :, 0:512]),
            "kT0": np.ascontiguousarray(kTp[:, :, 0:512]),
            "vT0": np.ascontiguousarray(vTp[:, :, 0:512]),
            "wq": _pmajor(Wq).astype(np_in),
            "wk": _pmajor(Wk).astype(np_in),
            "wv": _pmajor(Wv).astype(np_in),
        }
        if apply_mask:
            p = np.arange(128)[:, None]
            cc = np.arange(512)[None, :]
            dm = np.zeros((128, 2, 512), np.float32)
            dm[:, 0, :] = (cc >= p + 128 * h)
            dm[:, 1, :] = (cc >= p + 256 + 128 * h)
            im["dmask"] = dm.astype(np_in)
        in_maps.append(im)
    return in_maps


def _combine(results):
    out = np.empty((B, T, H), np.float32)
    for b in range(B):
        s = results[2 * b]["out"] + results[2 * b + 1]["out"]
        out[b] = (s[:H] / s[H:H + 1]).T
    return out


def kernel(q, k, v, mask, Wq, Wk, Wv):
    from concourse.bass_utils import run_bass_kernel_spmd
    import ml_dtypes

    q = np.ascontiguousarray(q, np.float32)
    k = np.ascontiguousarray(k, np.float32)
    v = np.ascontiguousarray(v, np.float32)
    Wq = np.ascontiguousarray(Wq, np.float32)
    Wk = np.ascontiguousarray(Wk, np.float32)
    Wv = np.ascontiguousarray(Wv, np.float32)
    mask = np.asarray(mask)

    # fast path is causal-only: its group loop projects KV chunks just
    # ahead of their first (causal) use, which would deadlock the in-order
    # PE queue for masks that attend ahead of the diagonal
    is_tril = bool((mask == np.tril(np.ones((T, T), mask.dtype))).all())
    if not is_tril:
        return _numpy_fallback(q, k, v, mask, Wq, Wk, Wv)

    np_in = ml_dtypes.bfloat16
    counts = [2 * j + 2 for j in range(NG)]
    nc = _get_program(("v12", True), counts, True)

    in_maps = _make_in_maps(q, k, v, mask, Wq, Wk, Wv, True, np_in)
    res = run_bass_kernel_spmd(nc, in_maps, list(range(8)))
    return _combine(res.results)
